# revision 1
# baseline (speedup 1.0000x reference)
"""Causal self-attention kernel for Trainium2, 8 NeuronCores.

Reference computation (per batch b):
    qkv = x @ w_attn.T + b_attn          [T, 3C]
    q,k,v split, per-head causal softmax(q k^T / sqrt(D)) @ v
    out = y @ w_proj.T + b_proj          [T, C]

Sharding (8 cores): 2D (batch=4) x (head-group=2).  Core c handles batch
b = c//2 and heads [8*(c%2), 8*(c%2)+8).  Each core computes a partial
projection output (contraction over its 512 head-dims); the host sums the
two partials per batch and adds b_proj (the cheap "all-reduce").

Device kernel layout choices:
  - All matmul operands are host-pre-transposed so every contraction dim
    lands on SBUF partitions: xT [C,T], w*T per head-pair, w_projT.
  - Attention computes S^T = k q^T ([tk, tq] layout) so the PV matmul
    (y~^T = v^T-stationary @ P^T) directly yields y^T, which feeds the
    projection matmul as the stationary operand.
  - softmax skips the max-subtraction (inputs are ~N(0,1) after the 1/8
    scale -- exp cannot overflow fp32) and folds the 1/sqrt(D) scale into
    the ACT Exp.  The denominator comes from a ones-column appended to v.
  - causal masking: block-skip for fully-masked blocks, a multiplicative
    {0,1} mask (affine_select-generated) for diagonal blocks.
  - mmdt_name="bf16x3": QKV/S/proj matmuls run as 3-pass bf16 splits
    (hi*hi + hi*lo + lo*hi, error ~2^-16) at 1 cycle/row instead of
    fp32's 4 cycles/row.  PV stays fp32 (splitting P^T costs too much
    DVE).  x / w_attn / w_proj are split hi/lo on the host.
"""

import numpy as np


def _import_concourse():
    try:
        import concourse.bass  # noqa: F401
    except ImportError:
        import sys
        for p in ("/opt/trn_rl_repo", "/root/.axon_site/_ro/trn_rl_repo"):
            if p not in sys.path:
                sys.path.insert(0, p)
    import concourse.bass as bass
    import concourse.tile as tile
    from concourse import bacc, bass_utils, mybir
    return bass, bacc, tile, mybir, bass_utils


B, T, C, H, D = 4, 2048, 1024, 16, 64
NCORES = 8
HEADS_PER_CORE = 8
NPAIR = HEADS_PER_CORE // 2


def build_attn_nc(*, T, C, NPAIR, COUT, D=64, TQ=512, mmdt_name="float32",
                  debug_taps=False, reps=1, pv_split=None, pdepth=2,
                  sps_bufs=3, mm_bufs=3, pt_bufs=3):
    """Build the per-core Bass program."""
    bass, bacc, tile, mybir, _ = _import_concourse()
    from concourse.tile import TileContext

    f32 = mybir.dt.float32
    bf16 = mybir.dt.bfloat16
    split3 = (mmdt_name == "bf16x3")
    if pv_split is None:
        pv_split = split3
    CH = C // 128          # contraction chunks
    NT = T // 128          # t chunks of 128 (tk chunks / v tiles / out rows)
    NQ = T // TQ           # q tiles
    NW = min(512, COUT)    # proj output column tile width
    NN = COUT // NW        # proj output column tiles
    F2 = 2 * D             # 128, per-pair q/k feature rows
    V2W = 2 * (D + 1)      # 130, v2 row width incl. ones columns
    scale = 1.0 / float(np.sqrt(D))
    MOFF = 128 * (TQ // 128 - 1)   # causal mask slice range
    # split passes (stationary_plane, moving_plane), ordered for LDW reuse
    PASSES = [(0, 0), (0, 1), (1, 0)] if split3 else [(0, 0)]
    NPL = 2 if split3 else 1
    wdt = bf16 if split3 else f32
    PL = ["_hi", "_lo"] if split3 else [""]

    nc = bacc.Bacc(None)

    xT_d = [nc.dram_tensor(f"xT{s}", [CH, 128, T], wdt, kind="ExternalInput")
            for s in PL]
    wq_d = [nc.dram_tensor(f"wq2{s}", [NPAIR, 128, CH * F2], wdt, kind="ExternalInput")
            for s in PL]
    wk_d = [nc.dram_tensor(f"wk2{s}", [NPAIR, 128, CH * F2], wdt, kind="ExternalInput")
            for s in PL]
    wv_d = [nc.dram_tensor(f"wv2{s}", [NPAIR, 128, CH * F2], wdt, kind="ExternalInput")
            for s in PL]
    wp_d = [nc.dram_tensor(f"wpT{s}", [NPAIR, 128, COUT], wdt, kind="ExternalInput")
            for s in PL]
    bq_d = nc.dram_tensor("bq2", [NPAIR, F2], f32, kind="ExternalInput")
    bk_d = nc.dram_tensor("bk2", [NPAIR, F2], f32, kind="ExternalInput")
    bv_d = nc.dram_tensor("bv2", [NPAIR, F2], f32, kind="ExternalInput")
    out_d = nc.dram_tensor("out", [T, COUT], f32, kind="ExternalOutput")
    if debug_taps:
        dbg_q = nc.dram_tensor("dbg_q", [NPAIR, 128, T], f32, kind="ExternalOutput")
        dbg_k = nc.dram_tensor("dbg_k", [NPAIR, 128, T], f32, kind="ExternalOutput")
        dbg_v = nc.dram_tensor("dbg_v", [NPAIR, 128, NT * V2W], f32, kind="ExternalOutput")
        dbg_y = nc.dram_tensor("dbg_y", [NPAIR, 128, T], f32, kind="ExternalOutput")
        dbg_r = nc.dram_tensor("dbg_r", [NPAIR, 128, T], f32, kind="ExternalOutput")

    with TileContext(nc) as tc:
        with (
            tc.tile_pool(name="persist", bufs=1) as persist,
            tc.tile_pool(name="wpool", bufs=1) as wpool,
            tc.tile_pool(name="qk", bufs=1) as qkpool,
            tc.tile_pool(name="pt", bufs=pt_bufs) as ptpool,
            tc.tile_pool(name="outp", bufs=2) as outpool,
            tc.tile_pool(name="ps", bufs=2, space="PSUM") as ps,
        ):
            def _emit():
                # ---- persistent tiles ---------------------------------
                xT = [[persist.tile([128, T], wdt, name=f"xT{s}{c}")
                       for c in range(CH)] for s in PL]
                for i in range(NPL):
                    for c in range(CH):
                        nc.sync.dma_start(out=xT[i][c], in_=xT_d[i][c])

                # wide causal mask (additive): 0 iff g >= p + MOFF else -1e30
                maskw = persist.tile([128, TQ + MOFF], f32, name="maskw")
                nc.gpsimd.memset(maskw, 0.0)
                nc.gpsimd.affine_select(
                    out=maskw, in_=maskw,
                    compare_op=mybir.AluOpType.is_ge, fill=-1e30,
                    base=-MOFF, channel_multiplier=-1,
                    pattern=[[1, TQ + MOFF]],
                )

                ident = persist.tile([128, 128], f32, name="ident")
                nc.gpsimd.memset(ident, 0.0)
                nc.gpsimd.affine_select(
                    out=ident, in_=ident,
                    compare_op=mybir.AluOpType.not_equal, fill=1.0,
                    base=0, channel_multiplier=1, pattern=[[-1, 128]],
                )
                yT = [persist.tile([128, T], f32, name=f"yT{p}")
                      for p in range(NPAIR)]
                wpT = [[persist.tile([128, COUT], wdt, name=f"wpT{s}{p}")
                        for p in range(NPAIR)] for s in PL]

                for p in range(NPAIR):
                    # ---- load pair weights + biases -------------------
                    wq = [wpool.tile([128, CH, F2], wdt, tag=f"wq{s}", name=f"wq{s}")
                          for s in PL]
                    wk = [wpool.tile([128, CH, F2], wdt, tag=f"wk{s}", name=f"wk{s}")
                          for s in PL]
                    wv = [wpool.tile([128, CH, F2], wdt, tag=f"wv{s}", name=f"wv{s}")
                          for s in PL]
                    for i in range(NPL):
                        nc.sync.dma_start(
                            out=wq[i], in_=wq_d[i][p].rearrange("P (c f) -> P c f", c=CH))
                        nc.sync.dma_start(
                            out=wk[i], in_=wk_d[i][p].rearrange("P (c f) -> P c f", c=CH))
                        nc.sync.dma_start(
                            out=wv[i], in_=wv_d[i][p].rearrange("P (c f) -> P c f", c=CH))
                    bq = wpool.tile([F2, 1], f32, tag="bq")
                    bk = wpool.tile([F2, 1], f32, tag="bk")
                    nc.sync.dma_start(out=bq, in_=bq_d[p].rearrange("(f o) -> f o", o=1))
                    nc.sync.dma_start(out=bk, in_=bk_d[p].rearrange("(f o) -> f o", o=1))
                    bv = wpool.tile([1, F2], f32, tag="bv")
                    nc.sync.dma_start(out=bv, in_=bv_d[p].rearrange("(o f) -> o f", o=1))
                    bvb = wpool.tile([128, F2], f32, tag="bvb")
                    nc.gpsimd.partition_broadcast(bvb, bv)

                    # ---- QKV ------------------------------------------
                    q2 = [qkpool.tile([128, T], wdt, tag=f"q2T{s}", name=f"q2T{s}")
                          for s in PL]
                    k2 = [qkpool.tile([128, T], wdt, tag=f"k2T{s}", name=f"k2T{s}")
                          for s in PL]
                    for jq in range(NQ):
                        jqs = slice(jq * TQ, (jq + 1) * TQ)
                        for dst, w, bias in ((q2, wq, bq), (k2, wk, bk)):
                            psq = ps.tile([128, TQ], f32, tag="mm", bufs=mm_bufs)
                            nmm = CH * len(PASSES)
                            i = 0
                            for c in range(CH):
                                for (si, mi) in PASSES:
                                    nc.tensor.matmul(
                                        psq, w[si][:, c, :], xT[mi][c][:, jqs],
                                        start=(i == 0), stop=(i == nmm - 1))
                                    i += 1
                            if split3:
                                tmp = ptpool.tile([128, TQ], f32, tag="qtmp", bufs=2)
                                nc.vector.tensor_scalar_add(tmp, psq, bias)
                                nc.vector.tensor_copy(dst[0][:, jqs], tmp)
                                nc.vector.tensor_sub(dst[1][:, jqs], tmp, dst[0][:, jqs])
                            else:
                                nc.vector.tensor_scalar_add(dst[0][:, jqs], psq, bias)

                    vdt = bf16 if pv_split else f32
                    v2 = [qkpool.tile([128, NT, V2W], vdt, tag=f"v2{s_}",
                                      name=f"v2{s_}")
                          for s_ in (PL if pv_split else PL[:1])]
                    nc.vector.memset(v2[0], 1.0)
                    if pv_split:
                        nc.vector.memset(v2[1], 0.0)
                    for jt in range(T // 512):
                        psvt = ps.tile([128, 512], f32, tag="mm", bufs=mm_bufs)
                        nmm = CH * len(PASSES)
                        i = 0
                        for c in range(CH):
                            for (si, mi) in PASSES:
                                nc.tensor.matmul(
                                    psvt, wv[si][:, c, :],
                                    xT[mi][c][:, jt * 512:(jt + 1) * 512],
                                    start=(i == 0), stop=(i == nmm - 1))
                                i += 1
                        vts = ptpool.tile([128, 512], f32, tag="vts", bufs=2)
                        nc.vector.tensor_copy(vts, psvt)
                        for sub in range(4):
                            it = jt * 4 + sub
                            psv = ps.tile([128, F2], f32, tag="mm", bufs=mm_bufs,
                                          name="psv")
                            nc.tensor.transpose(
                                psv, vts[:, sub * 128:(sub + 1) * 128], ident)
                            if pv_split:
                                tmpv = ptpool.tile([128, F2], f32, tag="tmpv", bufs=2)
                                nc.vector.tensor_add(tmpv, psv, bvb)
                                nc.vector.tensor_copy(v2[0][:, it, 0:D], tmpv[:, 0:D])
                                nc.vector.tensor_copy(
                                    v2[0][:, it, D + 1:2 * D + 1], tmpv[:, D:F2])
                                nc.vector.tensor_sub(
                                    v2[1][:, it, 0:D], tmpv[:, 0:D], v2[0][:, it, 0:D])
                                nc.vector.tensor_sub(
                                    v2[1][:, it, D + 1:2 * D + 1], tmpv[:, D:F2],
                                    v2[0][:, it, D + 1:2 * D + 1])
                            else:
                                nc.vector.tensor_copy(v2[0][:, it, 0:D], psv[:, 0:D])
                                nc.vector.tensor_copy(
                                    v2[0][:, it, D + 1:2 * D + 1], psv[:, D:F2])
                                nc.vector.tensor_add(
                                    v2[0][:, it, 0:D], v2[0][:, it, 0:D], bvb[:, 0:D])
                                nc.vector.tensor_add(
                                    v2[0][:, it, D + 1:2 * D + 1],
                                    v2[0][:, it, D + 1:2 * D + 1], bvb[:, D:F2])

                    if debug_taps and not split3:
                        nc.sync.dma_start(out=dbg_q[p], in_=q2[0])
                        nc.sync.dma_start(out=dbg_k[p], in_=k2[0])
                        nc.sync.dma_start(out=dbg_v[p],
                                          in_=v2[0].rearrange("P a b -> P (a b)"))

                    # ---- attention ------------------------------------
                    r2 = outpool.tile([128, T], f32, tag="r2", bufs=1)
                    for jq in range(NQ):
                        jqs = slice(jq * TQ, (jq + 1) * TQ)
                        ik_hi = min(NT - 1, (jq * TQ + TQ - 1) // 128)
                        actives = list(range(ik_hi + 1))
                        pvps = [ps.tile([D + 1, TQ], f32, tag=f"pv{h}", bufs=1,
                                        name=f"pv{h}")
                                for h in range(2)]
                        pending = []  # (ik, h, s_psum)

                        def flush(pend):
                            for (ik, h, sps) in pend:
                                r = ik - (jq * TQ) // 128
                                if r >= 0:
                                    nc.vector.tensor_add(
                                        sps, sps,
                                        maskw[:, MOFF - 128 * r: MOFF - 128 * r + TQ])
                                hsl = slice((D + 1) * h, (D + 1) * (h + 1))
                                if pv_split:
                                    pth = ptpool.tile([128, TQ], bf16, tag="pth")
                                    ptf = ptpool.tile([128, TQ], f32, tag="ptf")
                                    ptl = ptpool.tile([128, TQ], bf16, tag="ptl")
                                    nc.scalar.activation(
                                        pth, sps, mybir.ActivationFunctionType.Exp,
                                        scale=scale)
                                    nc.scalar.activation(
                                        ptf, sps, mybir.ActivationFunctionType.Exp,
                                        scale=scale)
                                    nc.vector.tensor_sub(ptl, ptf, pth)
                                    ptpl = [pth, ptl]
                                    PVP = [(0, 0), (0, 1), (1, 0)]
                                    nmm = len(PVP)
                                    for j, (si, mi) in enumerate(PVP):
                                        nc.tensor.matmul(
                                            pvps[h], v2[si][:, ik, hsl], ptpl[mi],
                                            start=(ik == 0 and j == 0),
                                            stop=(ik == actives[-1] and j == nmm - 1))
                                else:
                                    pt = ptpool.tile([128, TQ], f32, tag="pt")
                                    nc.scalar.activation(
                                        pt, sps, mybir.ActivationFunctionType.Exp,
                                        scale=scale)
                                    nc.tensor.matmul(
                                        pvps[h], v2[0][:, ik, hsl], pt,
                                        start=(ik == 0), stop=(ik == actives[-1]))

                        for ik in actives:
                            iks = slice(ik * 128, (ik + 1) * 128)
                            for h in range(2):
                                hs = slice(D * h, D * (h + 1))
                                sps = ps.tile([128, TQ], f32, tag="sps", bufs=sps_bufs)
                                i = 0
                                for (si, mi) in PASSES:
                                    nc.tensor.matmul(
                                        sps, k2[si][hs, iks], q2[mi][hs, jqs],
                                        start=(i == 0), stop=(i == len(PASSES) - 1))
                                    i += 1
                                pending.append((ik, h, sps))
                            if len(pending) > pdepth:
                                flush(pending[:-pdepth])
                                pending = pending[-pdepth:]
                        flush(pending)

                        for h in range(2):
                            tq = slice(jq * TQ, (jq + 1) * TQ)
                            nc.vector.tensor_copy(
                                yT[p][64 * h:64 * h + D, tq], pvps[h][0:D, :])
                            lst = ptpool.tile([1, TQ], f32, tag="lst", bufs=2)
                            nc.vector.tensor_copy(lst, pvps[h][D:D + 1, :])
                            rtmp = ptpool.tile([64, TQ], f32, tag="rtmp", bufs=2)
                            nc.gpsimd.partition_broadcast(rtmp, lst, channels=64)
                            nc.vector.tensor_copy(r2[64 * h:64 * (h + 1), tq], rtmp)

                    nc.vector.reciprocal(r2, r2)
                    nc.vector.tensor_mul(yT[p], yT[p], r2)
                    if debug_taps:
                        nc.sync.dma_start(out=dbg_r[p], in_=r2)
                        nc.sync.dma_start(out=dbg_y[p], in_=yT[p])

                # ---- projection ---------------------------------------
                for i in range(NPL):
                    for p in range(NPAIR):
                        nc.sync.dma_start(out=wpT[i][p], in_=wp_d[i][p])
                for it in range(NT):
                    its = slice(it * 128, (it + 1) * 128)
                    ot = outpool.tile([128, COUT], f32, tag="ot")
                    pps = [ps.tile([128, NW], f32, tag="mm", bufs=mm_bufs, name=f"pp{n}")
                           for n in range(NN)]
                    nmm = NPAIR * len(PASSES) * NN
                    i = 0
                    for p in range(NPAIR):
                        if split3:
                            yhi = ptpool.tile([128, 128], bf16, tag="yhi", bufs=2)
                            ylo = ptpool.tile([128, 128], bf16, tag="ylo", bufs=2)
                            nc.vector.tensor_copy(yhi, yT[p][:, its])
                            nc.vector.tensor_sub(ylo, yT[p][:, its], yhi)
                            ypl = [yhi, ylo]
                        else:
                            ypl = [yT[p][:, its]]
                        for (si, mi) in PASSES:
                            for n in range(NN):
                                nc.tensor.matmul(
                                    pps[n], ypl[si],
                                    wpT[mi][p][:, n * NW:(n + 1) * NW],
                                    start=(i // NN == 0),
                                    stop=(i // NN == NPAIR * len(PASSES) - 1))
                                i += 1
                    for n in range(NN):
                        nc.vector.tensor_copy(ot[:, n * NW:(n + 1) * NW], pps[n])
                    nc.sync.dma_start(out=out_d[its, :], in_=ot)

            if reps > 1:
                with tc.For_i(0, reps, 1):
                    _emit()
            else:
                _emit()

    nc.finalize()
    return nc


def _split_hi_lo(a):
    import ml_dtypes
    hi = a.astype(ml_dtypes.bfloat16)
    lo = (a - hi.astype(np.float32)).astype(ml_dtypes.bfloat16)
    return hi, lo


def shard_inputs(x, w_attn, b_attn, w_proj, *, T=T, C=C, H=H, D=D,
                 ncores=NCORES, heads_per_core=HEADS_PER_CORE,
                 mmdt_name="float32"):
    """Host-side sharding + layout prep.  Returns list of per-core in_maps."""
    split3 = (mmdt_name == "bf16x3")
    npair = heads_per_core // 2
    CH = C // 128
    in_maps = []
    for core in range(ncores):
        b, g = core // 2, core % 2
        xT = np.ascontiguousarray(x[b].T).reshape(CH, 128, T)
        wq2 = np.empty((npair, 128, CH * 2 * D), np.float32)
        wk2 = np.empty_like(wq2)
        wv2 = np.empty_like(wq2)
        bq2 = np.empty((npair, 2 * D), np.float32)
        bk2 = np.empty_like(bq2)
        bv2 = np.zeros((npair, 2 * D), np.float32)
        for p in range(npair):
            ha = g * heads_per_core + 2 * p
            r0 = ha * D
            for dst, off in ((wq2, 0), (wk2, C), (wv2, 2 * C)):
                wpair = w_attn[off + r0: off + r0 + 2 * D, :]       # [128, C]
                dst[p] = (wpair.T.reshape(CH, 128, 2 * D)
                          .transpose(1, 0, 2).reshape(128, CH * 2 * D))
            bq2[p] = b_attn[r0: r0 + 2 * D]
            bk2[p] = b_attn[C + r0: C + r0 + 2 * D]
            bv2[p] = b_attn[2 * C + r0: 2 * C + r0 + 2 * D]
        cols = slice(g * heads_per_core * D, (g + 1) * heads_per_core * D)
        wpT = (np.ascontiguousarray(w_proj[:, cols].T)
               .reshape(npair, 128, w_proj.shape[0]))
        m = {"bq2": bq2, "bk2": bk2, "bv2": bv2}
        if split3:
            for name, arr in (("xT", xT), ("wq2", wq2), ("wk2", wk2),
                              ("wv2", wv2), ("wpT", wpT)):
                hi, lo = _split_hi_lo(np.ascontiguousarray(arr))
                m[name + "_hi"] = hi
                m[name + "_lo"] = lo
        else:
            m.update({"xT": np.ascontiguousarray(xT), "wq2": wq2, "wk2": wk2,
                      "wv2": wv2, "wpT": np.ascontiguousarray(wpT)})
        in_maps.append(m)
    return in_maps


_NC_CACHE = {}


def _get_nc(mmdt_name="float32"):
    if mmdt_name not in _NC_CACHE:
        _NC_CACHE[mmdt_name] = build_attn_nc(
            T=T, C=C, NPAIR=NPAIR, COUT=C, D=D, TQ=512, mmdt_name=mmdt_name)
    return _NC_CACHE[mmdt_name]


MMDT = "bf16x3"


def kernel(x, w_attn, b_attn, w_proj, b_proj):
    _, _, _, _, bass_utils = _import_concourse()
    x = np.asarray(x, np.float32)
    w_attn = np.asarray(w_attn, np.float32)
    b_attn = np.asarray(b_attn, np.float32)
    w_proj = np.asarray(w_proj, np.float32)
    b_proj = np.asarray(b_proj, np.float32)

    nc = _get_nc(MMDT)
    in_maps = shard_inputs(x, w_attn, b_attn, w_proj, mmdt_name=MMDT)
    res = bass_utils.run_bass_kernel_spmd(nc, in_maps, core_ids=list(range(NCORES)))
    out = np.empty((B, T, C), np.float32)
    for b in range(B):
        out[b] = res.results[2 * b]["out"] + res.results[2 * b + 1]["out"] + b_proj
    return out



# revision 3
# speedup vs baseline: 2.1391x; 2.1391x over previous
"""Causal self-attention kernel for Trainium2, 8 NeuronCores.

Reference computation (per batch b):
    qkv = x @ w_attn.T + b_attn          [T, 3C]
    q,k,v split, per-head causal softmax(q k^T / sqrt(D)) @ v
    out = y @ w_proj.T + b_proj          [T, C]

Sharding (8 cores): 2D (batch=4) x (head-group=2).  Core c handles batch
b = c//2 and heads [8*(c%2), 8*(c%2)+8).  Each core computes a partial
projection output (contraction over its 512 head-dims); the host sums the
two partials per batch and adds b_proj (the cheap "all-reduce").

Device kernel design (single-pass bf16 matmuls; rel-err budget 2e-2 vs
measured ~4e-3 for full bf16):
  - All matmul operands are host-pre-transposed so every contraction dim
    lands on SBUF partitions: xT [C,T], wq/wk per head-pair, w_projT.
  - Attention computes S^T = k q^T ([tk, tq] layout) so the PV matmul
    (y~^T = v-stationary @ P^T) directly yields y^T, which feeds the
    projection matmul as the stationary operand.
  - v is produced directly in [token, dim] layout by using xT chunks as
    the *stationary* operand (out partitions = tokens), so no PE
    transposes are needed.
  - v2 layout per tk-block: cols [0:64]=v_h0, [64:128]=ones, [128:192]=
    v_h1.  The PV stationary for h0 is cols [0:128] and for h1 cols
    [64:192]; the 64 ones-columns make 64 PSUM partitions hold the
    softmax denominator, broadcast for free.  Normalization is then one
    DVE reciprocal + one multiply per (q-tile, head).
  - softmax skips the max-subtraction (scores are ~N(0,1) after the
    1/sqrt(D) scale -- exp cannot overflow fp32) and folds the scale
    into the ACT Exp, which writes bf16 P tiles directly.
  - causal masking: fully-masked 128-col subblocks are skipped by
    restricting S/exp/PV streams to the valid column range; diagonal
    128x128 blocks get an additive -1e30 triangular mask on DVE.
  - partial projection outputs are written in bf16 (halves output DMA);
    host accumulates in fp32.
"""

import numpy as np


def _import_concourse():
    try:
        import concourse.bass  # noqa: F401
    except ImportError:
        import sys
        for p in ("/opt/trn_rl_repo", "/root/.axon_site/_ro/trn_rl_repo"):
            if p not in sys.path:
                sys.path.insert(0, p)
    import concourse.bass as bass
    import concourse.tile as tile
    from concourse import bacc, bass_utils, mybir
    return bass, bacc, tile, mybir, bass_utils


B, T, C, H, D = 4, 2048, 1024, 16, 64
NCORES = 8
HEADS_PER_CORE = 8
NPAIR = HEADS_PER_CORE // 2


def build_attn_nc(*, T, C, NPAIR, COUT, D=64, TQ=512, mmdt_name="bf16",
                  reps=1, pdepth=2, sps_bufs=3, mm_bufs=3, pt_bufs=3):
    """Build the per-core Bass program (single-pass bf16 design)."""
    bass, bacc, tile, mybir, _ = _import_concourse()
    from concourse.tile import TileContext

    assert mmdt_name == "bf16"
    f32 = mybir.dt.float32
    bf16 = mybir.dt.bfloat16
    CH = C // 128          # contraction chunks of x / w_attn rows
    NT = T // 128          # tk blocks / v token blocks / out row tiles
    NQ = T // TQ           # q tiles
    NW = min(512, COUT)    # proj output column tile width
    NN = COUT // NW        # proj output column tiles
    F2 = 2 * D             # 128, per-pair q/k feature rows
    VW = 3 * D             # 192, v2 row width: v_h0 | ones | v_h1
    RB = TQ // 128         # 128-col subblocks per q tile
    scale = 1.0 / float(np.sqrt(D))

    nc = bacc.Bacc(None)

    xT_d = nc.dram_tensor("xT", [CH, 128, T], bf16, kind="ExternalInput")
    wq_d = nc.dram_tensor("wq2", [NPAIR, 128, CH * F2], bf16, kind="ExternalInput")
    wk_d = nc.dram_tensor("wk2", [NPAIR, 128, CH * F2], bf16, kind="ExternalInput")
    wv_d = nc.dram_tensor("wv2", [NPAIR, 128, CH * F2], bf16, kind="ExternalInput")
    wp_d = nc.dram_tensor("wpT", [NPAIR, 128, COUT], bf16, kind="ExternalInput")
    bq_d = nc.dram_tensor("bq2", [NPAIR, F2], f32, kind="ExternalInput")
    bk_d = nc.dram_tensor("bk2", [NPAIR, F2], f32, kind="ExternalInput")
    bv_d = nc.dram_tensor("bv2", [NPAIR, F2], f32, kind="ExternalInput")
    out_d = nc.dram_tensor("out", [T, COUT], bf16, kind="ExternalOutput")

    with TileContext(nc) as tc:
        with (
            tc.tile_pool(name="persist", bufs=1) as persist,
            tc.tile_pool(name="wpool", bufs=2) as wpool,
            tc.tile_pool(name="qk", bufs=2) as qkpool,
            tc.tile_pool(name="pt", bufs=pt_bufs) as ptpool,
            tc.tile_pool(name="outp", bufs=2) as outpool,
            tc.tile_pool(name="ps", bufs=2, space="PSUM") as ps,
        ):
            def _emit():
                # ---- persistent tiles ---------------------------------
                xT = [persist.tile([128, T], bf16, name=f"xT{c}")
                      for c in range(CH)]
                for c in range(CH):
                    nc.sync.dma_start(out=xT[c], in_=xT_d[c])
                wpT = [persist.tile([128, COUT], bf16, name=f"wpT{p}")
                       for p in range(NPAIR)]
                for p in range(NPAIR):
                    nc.sync.dma_start(out=wpT[p], in_=wp_d[p])

                # 128x128 additive causal mask: 0 iff col >= row else -1e30
                tri = persist.tile([128, 128], f32, name="tri")
                nc.gpsimd.memset(tri, 0.0)
                nc.gpsimd.affine_select(
                    out=tri, in_=tri,
                    compare_op=mybir.AluOpType.is_ge, fill=-1e30,
                    base=0, channel_multiplier=-1,
                    pattern=[[1, 128]],
                )

                yTb = [persist.tile([128, T], bf16, name=f"yTb{p}")
                       for p in range(NPAIR)]

                for p in range(NPAIR):
                    # ---- load pair weights + biases -------------------
                    wq = wpool.tile([128, CH, F2], bf16, tag="wq", name="wq")
                    wk = wpool.tile([128, CH, F2], bf16, tag="wk", name="wk")
                    wv = wpool.tile([128, CH, F2], bf16, tag="wv", name="wv")
                    nc.sync.dma_start(
                        out=wq, in_=wq_d[p].rearrange("P (c f) -> P c f", c=CH))
                    nc.sync.dma_start(
                        out=wk, in_=wk_d[p].rearrange("P (c f) -> P c f", c=CH))
                    nc.sync.dma_start(
                        out=wv, in_=wv_d[p].rearrange("P (c f) -> P c f", c=CH))
                    bq = wpool.tile([F2, 1], f32, tag="bq")
                    bk = wpool.tile([F2, 1], f32, tag="bk")
                    nc.sync.dma_start(out=bq, in_=bq_d[p].rearrange("(f o) -> f o", o=1))
                    nc.sync.dma_start(out=bk, in_=bk_d[p].rearrange("(f o) -> f o", o=1))
                    bv = wpool.tile([1, F2], f32, tag="bv")
                    nc.sync.dma_start(out=bv, in_=bv_d[p].rearrange("(o f) -> o f", o=1))
                    bvb = wpool.tile([128, F2], f32, tag="bvb")
                    nc.gpsimd.partition_broadcast(bvb, bv)

                    # ---- Q, K:  [F2, T] = w^T-stationary @ xT ---------
                    q2 = qkpool.tile([128, T], bf16, tag="q2T", name="q2T")
                    k2 = qkpool.tile([128, T], bf16, tag="k2T", name="k2T")
                    for jq in range(NQ):
                        jqs = slice(jq * TQ, (jq + 1) * TQ)
                        for dst, w, bias in ((q2, wq, bq), (k2, wk, bk)):
                            psq = ps.tile([128, TQ], f32, tag="mm", bufs=mm_bufs)
                            for c in range(CH):
                                nc.tensor.matmul(
                                    psq, w[:, c, :], xT[c][:, jqs],
                                    start=(c == 0), stop=(c == CH - 1))
                            nc.vector.tensor_scalar_add(dst[:, jqs], psq, bias)

                    # ---- V: [tok, d] via xT-stationary ----------------
                    v2 = qkpool.tile([128, NT, VW], bf16, tag="v2", name="v2")
                    if p < 2:
                        # ones block survives in this rotating buffer for
                        # later pairs (v-cols are fully rewritten, ones not)
                        nc.gpsimd.memset(v2[:, :, D:2 * D], 1.0)
                    for it in range(NT):
                        its = slice(it * 128, (it + 1) * 128)
                        psv = ps.tile([128, TQ], f32, tag="mm", bufs=mm_bufs,
                                      name="psv")
                        for c in range(CH):
                            nc.tensor.matmul(
                                psv[:, 0:F2], xT[c][:, its], wv[:, c, :],
                                start=(c == 0), stop=(c == CH - 1))
                        nc.vector.tensor_add(
                            v2[:, it, 0:D], psv[:, 0:D], bvb[:, 0:D])
                        nc.vector.tensor_add(
                            v2[:, it, 2 * D:VW], psv[:, D:F2], bvb[:, D:F2])

                    # ---- attention ------------------------------------
                    for jq in range(NQ):
                        ik_hi = min(NT - 1, (jq * TQ + TQ - 1) // 128)
                        actives = list(range(ik_hi + 1))
                        pvps = [ps.tile([128, TQ], f32, tag=f"pv{h}", bufs=1,
                                        name=f"pv{h}")
                                for h in range(2)]
                        pending = []  # (ik, h, s_psum, col_lo)

                        def flush(pend):
                            for (ik, h, sps, col_lo) in pend:
                                r = ik - jq * RB
                                if r >= 0:
                                    nc.vector.tensor_add(
                                        sps[:, 128 * r:128 * (r + 1)],
                                        sps[:, 128 * r:128 * (r + 1)], tri)
                                pt = ptpool.tile([128, TQ], bf16, tag="pt")
                                nc.scalar.activation(
                                    pt[:, col_lo:TQ], sps[:, col_lo:TQ],
                                    mybir.ActivationFunctionType.Exp,
                                    scale=scale)
                                vsl = slice(D * h, D * h + F2)
                                nc.tensor.matmul(
                                    pvps[h][:, col_lo:TQ], v2[:, ik, vsl],
                                    pt[:, col_lo:TQ],
                                    start=(ik == 0), stop=(ik == actives[-1]))

                        for ik in actives:
                            iks = slice(ik * 128, (ik + 1) * 128)
                            col_lo = max(0, 128 * (ik - jq * RB))
                            for h in range(2):
                                hs = slice(D * h, D * (h + 1))
                                sps = ps.tile([128, TQ], f32, tag="sps",
                                              bufs=sps_bufs)
                                nc.tensor.matmul(
                                    sps[:, col_lo:TQ], k2[hs, iks],
                                    q2[hs, jq * TQ + col_lo:(jq + 1) * TQ],
                                    start=True, stop=True)
                                pending.append((ik, h, sps, col_lo))
                            if len(pending) > pdepth:
                                flush(pending[:-pdepth])
                                pending = pending[-pdepth:]
                        flush(pending)

                        # normalize: h0 y=rows[0:64]/rows[64:128],
                        #            h1 y=rows[64:128]/rows[0:64]
                        tq = slice(jq * TQ, (jq + 1) * TQ)
                        rd0 = ptpool.tile([D, TQ], f32, tag="rd0", bufs=2)
                        nc.vector.reciprocal(rd0, pvps[0][D:F2, :])
                        nc.vector.tensor_mul(yTb[p][0:D, tq], pvps[0][0:D, :], rd0)
                        rd1 = ptpool.tile([D, TQ], f32, tag="rd1", bufs=2)
                        nc.vector.reciprocal(rd1, pvps[1][0:D, :])
                        nc.vector.tensor_mul(yTb[p][D:F2, tq], pvps[1][D:F2, :], rd1)

                # ---- projection ---------------------------------------
                for it in range(NT):
                    its = slice(it * 128, (it + 1) * 128)
                    ot = outpool.tile([128, COUT], bf16, tag="ot")
                    pps = [ps.tile([128, NW], f32, tag="mm", bufs=mm_bufs,
                                   name=f"pp{n}")
                           for n in range(NN)]
                    for p in range(NPAIR):
                        for n in range(NN):
                            nc.tensor.matmul(
                                pps[n], yTb[p][:, its],
                                wpT[p][:, n * NW:(n + 1) * NW],
                                start=(p == 0), stop=(p == NPAIR - 1))
                    for n in range(NN):
                        nc.scalar.copy(ot[:, n * NW:(n + 1) * NW], pps[n])
                    nc.sync.dma_start(out=out_d[its, :], in_=ot)

            if reps > 1:
                with tc.For_i(0, reps, 1):
                    _emit()
            else:
                _emit()

    nc.finalize()
    return nc


def shard_inputs(x, w_attn, b_attn, w_proj, *, T=T, C=C, H=H, D=D,
                 ncores=NCORES, heads_per_core=HEADS_PER_CORE,
                 mmdt_name="bf16"):
    """Host-side sharding + layout prep.  Returns list of per-core in_maps."""
    import ml_dtypes
    bf16 = ml_dtypes.bfloat16
    npair = heads_per_core // 2
    CH = C // 128
    in_maps = []
    for core in range(ncores):
        b, g = core // 2, core % 2
        xT = np.ascontiguousarray(x[b].T).reshape(CH, 128, T)
        wq2 = np.empty((npair, 128, CH * 2 * D), np.float32)
        wk2 = np.empty_like(wq2)
        wv2 = np.empty_like(wq2)
        bq2 = np.empty((npair, 2 * D), np.float32)
        bk2 = np.empty_like(bq2)
        bv2 = np.zeros((npair, 2 * D), np.float32)
        for p in range(npair):
            ha = g * heads_per_core + 2 * p
            r0 = ha * D
            for dst, off in ((wq2, 0), (wk2, C), (wv2, 2 * C)):
                wpair = w_attn[off + r0: off + r0 + 2 * D, :]       # [128, C]
                dst[p] = (wpair.T.reshape(CH, 128, 2 * D)
                          .transpose(1, 0, 2).reshape(128, CH * 2 * D))
            bq2[p] = b_attn[r0: r0 + 2 * D]
            bk2[p] = b_attn[C + r0: C + r0 + 2 * D]
            bv2[p] = b_attn[2 * C + r0: 2 * C + r0 + 2 * D]
        cols = slice(g * heads_per_core * D, (g + 1) * heads_per_core * D)
        wpT = (np.ascontiguousarray(w_proj[:, cols].T)
               .reshape(npair, 128, w_proj.shape[0]))
        m = {
            "xT": xT.astype(bf16), "wq2": wq2.astype(bf16),
            "wk2": wk2.astype(bf16), "wv2": wv2.astype(bf16),
            "wpT": np.ascontiguousarray(wpT).astype(bf16),
            "bq2": bq2, "bk2": bk2, "bv2": bv2,
        }
        in_maps.append(m)
    return in_maps


_NC_CACHE = {}


def _get_nc(mmdt_name="bf16"):
    if mmdt_name not in _NC_CACHE:
        _NC_CACHE[mmdt_name] = build_attn_nc(
            T=T, C=C, NPAIR=NPAIR, COUT=C, D=D, TQ=512, mmdt_name=mmdt_name)
    return _NC_CACHE[mmdt_name]


MMDT = "bf16"


def kernel(x, w_attn, b_attn, w_proj, b_proj):
    _, _, _, _, bass_utils = _import_concourse()
    x = np.asarray(x, np.float32)
    w_attn = np.asarray(w_attn, np.float32)
    b_attn = np.asarray(b_attn, np.float32)
    w_proj = np.asarray(w_proj, np.float32)
    b_proj = np.asarray(b_proj, np.float32)

    nc = _get_nc(MMDT)
    in_maps = shard_inputs(x, w_attn, b_attn, w_proj, mmdt_name=MMDT)
    res = bass_utils.run_bass_kernel_spmd(nc, in_maps, core_ids=list(range(NCORES)))
    out = np.empty((B, T, C), np.float32)
    for b in range(B):
        out[b] = (res.results[2 * b]["out"].astype(np.float32)
                  + res.results[2 * b + 1]["out"].astype(np.float32) + b_proj)
    return out


# revision 12
# speedup vs baseline: 3.2401x; 1.5147x over previous
"""Causal self-attention kernel for Trainium2, 8 NeuronCores.

Reference computation (per batch b):
    qkv = x @ w_attn.T + b_attn          [T, 3C]
    q,k,v split, per-head causal softmax(q k^T / sqrt(D)) @ v
    out = y @ w_proj.T + b_proj          [T, C]

Sharding (8 cores): 2D (batch=4) x (head-group=2).  Core c handles batch
b = c//2 and heads [8*(c%2), 8*(c%2)+8).  Each core computes a partial
projection output (contraction over its 512 head-dims); the host sums the
two partials per batch and adds b_proj (the cheap "all-reduce").

Device kernel design (single-pass bf16 matmuls; rel-err budget 2e-2 vs
measured ~4e-3 for full bf16):
  - All matmul operands are host-pre-transposed so every contraction dim
    lands on SBUF partitions: xT [C,T], wq/wk per head-pair, w_projT.
  - Attention computes S^T = k q^T ([tk, tq] layout) so the PV matmul
    (y~^T = v-stationary @ P^T) directly yields y^T, which feeds the
    projection matmul as the stationary operand.
  - v is produced directly in [token, dim] layout by using xT chunks as
    the *stationary* operand (out partitions = tokens), so no PE
    transposes are needed.
  - v2 layout per tk-block: cols [0:64]=v_h0, [64:128]=ones, [128:192]=
    v_h1.  The PV stationary for h0 is cols [0:128] and for h1 cols
    [64:192]; the 64 ones-columns make 64 PSUM partitions hold the
    softmax denominator, broadcast for free.  Normalization is then one
    DVE reciprocal + one multiply per (q-tile, head).
  - softmax skips the max-subtraction (scores are ~N(0,1) after the
    1/sqrt(D) scale -- exp cannot overflow fp32) and folds the scale
    into the ACT Exp, which writes bf16 P tiles directly.
  - causal masking: fully-masked 128-col subblocks are skipped by
    restricting S/exp/PV streams to the valid column range; diagonal
    128x128 blocks get an additive -1e30 triangular mask on DVE.
  - partial projection outputs are written in bf16 (halves output DMA);
    host accumulates in fp32.
"""

import numpy as np


def _import_concourse():
    try:
        import concourse.bass  # noqa: F401
    except ImportError:
        import sys
        for p in ("/opt/trn_rl_repo", "/root/.axon_site/_ro/trn_rl_repo"):
            if p not in sys.path:
                sys.path.insert(0, p)
    import concourse.bass as bass
    import concourse.tile as tile
    from concourse import bacc, bass_utils, mybir
    return bass, bacc, tile, mybir, bass_utils


B, T, C, H, D = 4, 2048, 1024, 16, 64
NCORES = 8
HEADS_PER_CORE = 8
NPAIR = HEADS_PER_CORE // 2


def build_attn_nc(*, T, C, NPAIR, COUT, D=64, TQ=512, mmdt_name="bf16",
                  reps=1, pdepth=1, sps_bufs=2, mm_bufs=2, pt_bufs=3):
    """Build the per-core Bass program (single-pass bf16 design)."""
    bass, bacc, tile, mybir, _ = _import_concourse()
    from concourse.tile import TileContext

    assert mmdt_name == "bf16"
    f32 = mybir.dt.float32
    bf16 = mybir.dt.bfloat16
    CH = C // 128          # contraction chunks of x / w_attn rows
    NT = T // 128          # tk blocks / v token blocks / out row tiles
    NQ = T // TQ           # q tiles
    NW = min(512, COUT)    # proj output column tile width
    NN = COUT // NW        # proj output column tiles
    F2 = 2 * D             # 128, per-pair q/k feature rows
    VW = 3 * D             # 192, v2 row width: v_h0 | ones | v_h1
    RB = TQ // 128         # 128-col subblocks per q tile
    scale = 1.0 / float(np.sqrt(D))

    nc = bacc.Bacc(None)

    xT_d = nc.dram_tensor("xT", [CH, 128, T], bf16, kind="ExternalInput")
    wq_d = nc.dram_tensor("wq2", [NPAIR, 128, CH * F2], bf16, kind="ExternalInput")
    wk_d = nc.dram_tensor("wk2", [NPAIR, 128, CH * F2], bf16, kind="ExternalInput")
    wv_d = nc.dram_tensor("wv2", [NPAIR, 128, CH * F2], bf16, kind="ExternalInput")
    wp_d = nc.dram_tensor("wpT", [NPAIR, 128, COUT], bf16, kind="ExternalInput")
    # packed biases: column 2p = q bias, 2p+1 = k bias for pair p
    b2_d = nc.dram_tensor("b2", [F2, 2 * NPAIR], f32, kind="ExternalInput")
    # v bias as a row per pair, for partition_broadcast
    bvr_d = nc.dram_tensor("bvr", [1, NPAIR * F2], f32, kind="ExternalInput")
    out_d = nc.dram_tensor("out", [T, COUT], bf16, kind="ExternalOutput")

    with TileContext(nc) as tc:
        with (
            tc.tile_pool(name="persist", bufs=1) as persist,
            tc.tile_pool(name="wpool", bufs=2) as wpool,
            tc.tile_pool(name="qk", bufs=2) as qkpool,
            tc.tile_pool(name="pt", bufs=pt_bufs) as ptpool,
            tc.tile_pool(name="outp", bufs=2) as outpool,
            tc.tile_pool(name="ps", bufs=2, space="PSUM") as ps,
        ):
            def _emit():
                # ---- persistent tiles ---------------------------------
                xT = [persist.tile([128, T], bf16, name=f"xT{c}")
                      for c in range(CH)]
                for c in range(CH):
                    nc.sync.dma_start(out=xT[c], in_=xT_d[c])
                wpT = [persist.tile([128, COUT], bf16, name=f"wpT{p}")
                       for p in range(NPAIR)]
                for p in range(NPAIR):
                    nc.sync.dma_start(out=wpT[p], in_=wp_d[p])

                # multiplicative bf16 causal mask (both heads): 1 iff
                # col >= row else 0, duplicated along a middle dim of 2
                tri2 = persist.tile([128, 2, 128], bf16, name="tri2")
                nc.gpsimd.memset(tri2, 1.0)
                nc.gpsimd.affine_select(
                    out=tri2, in_=tri2,
                    compare_op=mybir.AluOpType.is_ge, fill=0.0,
                    base=0, channel_multiplier=-1,
                    pattern=[[0, 2], [1, 128]],
                )

                b2t = persist.tile([F2, 2 * NPAIR], f32, name="b2t")
                nc.sync.dma_start(out=b2t, in_=b2_d[:, :])
                bvr = persist.tile([1, NPAIR * F2], f32, name="bvr")
                nc.sync.dma_start(out=bvr, in_=bvr_d[:, :])

                yTb = [persist.tile([128, T], bf16, name=f"yTb{p}")
                       for p in range(NPAIR)]

                for p in range(NPAIR):
                    # ---- load pair weights + biases -------------------
                    wq = wpool.tile([128, CH * F2], bf16, tag="wq", name="wq")
                    wk = wpool.tile([128, CH * F2], bf16, tag="wk", name="wk")
                    wv = wpool.tile([128, CH * F2], bf16, tag="wv", name="wv")
                    nc.sync.dma_start(out=wq, in_=wq_d[p])
                    nc.sync.dma_start(out=wk, in_=wk_d[p])
                    nc.sync.dma_start(out=wv, in_=wv_d[p])
                    bq = b2t[:, 2 * p:2 * p + 1]
                    bk = b2t[:, 2 * p + 1:2 * p + 2]
                    bvb = wpool.tile([128, F2], f32, tag="bvb")
                    nc.gpsimd.partition_broadcast(
                        bvb, bvr[:, p * F2:(p + 1) * F2])

                    # ---- Q, K:  [F2, T] = w^T-stationary @ xT ---------
                    q2 = qkpool.tile([128, T], bf16, tag="q2T", name="q2T")
                    k2 = qkpool.tile([128, T], bf16, tag="k2T", name="k2T")
                    for jq in range(NQ):
                        jqs = slice(jq * TQ, (jq + 1) * TQ)
                        for dst, w, bias in ((q2, wq, bq), (k2, wk, bk)):
                            psq = ps.tile([128, TQ], f32, tag="mm", bufs=mm_bufs)
                            for c in range(CH):
                                nc.tensor.matmul(
                                    psq, w[:, c * F2:(c + 1) * F2], xT[c][:, jqs],
                                    start=(c == 0), stop=(c == CH - 1))
                            nc.vector.tensor_scalar_add(dst[:, jqs], psq, bias)

                    # ---- V: [tok, d] via xT-stationary ----------------
                    v2 = qkpool.tile([128, NT, VW], bf16, tag="v2", name="v2")
                    if p < 2:
                        # ones block survives in this rotating buffer for
                        # later pairs (v-cols are fully rewritten, ones not)
                        nc.gpsimd.memset(v2[:, :, D:2 * D], 1.0)
                    for it in range(NT):
                        its = slice(it * 128, (it + 1) * 128)
                        psv = ps.tile([128, TQ], f32, tag="mm", bufs=mm_bufs,
                                      name="psv")
                        for c in range(CH):
                            nc.tensor.matmul(
                                psv[:, 0:F2], xT[c][:, its],
                                wv[:, c * F2:(c + 1) * F2],
                                start=(c == 0), stop=(c == CH - 1))
                        nc.vector.tensor_add(
                            v2[:, it, 0:D], psv[:, 0:D], bvb[:, 0:D])
                        nc.vector.tensor_add(
                            v2[:, it, 2 * D:VW], psv[:, D:F2], bvb[:, D:F2])

                    # ---- attention ------------------------------------
                    for jq in range(NQ):
                        ik_hi = min(NT - 1, (jq * TQ + TQ - 1) // 128)
                        actives = list(range(ik_hi + 1))
                        pvps = [ps.tile([128, TQ], f32, tag=f"pv{h}", bufs=1,
                                        name=f"pv{h}")
                                for h in range(2)]
                        pending = []  # (ik, s_psum, col_lo)

                        def flush(pend):
                            for (ik, sps, col_lo) in pend:
                                r = ik - jq * RB
                                pt = ptpool.tile([128, 2, TQ], bf16, tag="pt")
                                nc.scalar.activation(
                                    pt[:, :, col_lo:TQ], sps[:, :, col_lo:TQ],
                                    mybir.ActivationFunctionType.Exp,
                                    scale=scale)
                                if r >= 0:
                                    nc.vector.tensor_mul(
                                        pt[:, :, 128 * r:128 * (r + 1)],
                                        pt[:, :, 128 * r:128 * (r + 1)], tri2)
                                for h in range(2):
                                    vsl = slice(D * h, D * h + F2)
                                    nc.tensor.matmul(
                                        pvps[h][:, col_lo:TQ], v2[:, ik, vsl],
                                        pt[:, h, col_lo:TQ],
                                        start=(ik == 0),
                                        stop=(ik == actives[-1]))

                        for ik in actives:
                            iks = slice(ik * 128, (ik + 1) * 128)
                            col_lo = max(0, 128 * (ik - jq * RB))
                            sps = ps.tile([128, 2, TQ], f32, tag="sps",
                                          bufs=sps_bufs)
                            for h in range(2):
                                hs = slice(D * h, D * (h + 1))
                                nc.tensor.matmul(
                                    sps[:, h, col_lo:TQ], k2[hs, iks],
                                    q2[hs, jq * TQ + col_lo:(jq + 1) * TQ],
                                    start=True, stop=True)
                            pending.append((ik, sps, col_lo))
                            if len(pending) > pdepth:
                                flush(pending[:-pdepth])
                                pending = pending[-pdepth:]
                        flush(pending)

                        # normalize: h0 y=rows[0:64]/rows[64:128],
                        #            h1 y=rows[64:128]/rows[0:64]
                        tq = slice(jq * TQ, (jq + 1) * TQ)
                        rd0 = ptpool.tile([D, TQ], f32, tag="rd0", bufs=2)
                        nc.vector.reciprocal(rd0, pvps[0][D:F2, :])
                        nc.vector.tensor_mul(yTb[p][0:D, tq], pvps[0][0:D, :], rd0)
                        rd1 = ptpool.tile([D, TQ], f32, tag="rd1", bufs=2)
                        nc.vector.reciprocal(rd1, pvps[1][0:D, :])
                        nc.vector.tensor_mul(yTb[p][D:F2, tq], pvps[1][D:F2, :], rd1)

                # ---- projection ---------------------------------------
                for it in range(NT):
                    its = slice(it * 128, (it + 1) * 128)
                    ot = outpool.tile([128, COUT], bf16, tag="ot")
                    pps = [ps.tile([128, NW], f32, tag="mm", bufs=mm_bufs,
                                   name=f"pp{n}")
                           for n in range(NN)]
                    for p in range(NPAIR):
                        for n in range(NN):
                            nc.tensor.matmul(
                                pps[n], yTb[p][:, its],
                                wpT[p][:, n * NW:(n + 1) * NW],
                                start=(p == 0), stop=(p == NPAIR - 1))
                    # split the PSUM->SBUF copies across the two idle engines
                    nc.scalar.copy(ot[:, 0:NW], pps[0])
                    for n in range(1, NN):
                        nc.vector.tensor_copy(ot[:, n * NW:(n + 1) * NW], pps[n])
                    nc.sync.dma_start(out=out_d[its, :], in_=ot)

            if reps > 1:
                with tc.For_i(0, reps, 1):
                    _emit()
            else:
                _emit()

    nc.finalize()
    return nc


def shard_inputs(x, w_attn, b_attn, w_proj, *, T=T, C=C, H=H, D=D,
                 ncores=NCORES, heads_per_core=HEADS_PER_CORE,
                 mmdt_name="bf16"):
    """Host-side sharding + layout prep.  Returns list of per-core in_maps."""
    import ml_dtypes
    bf16 = ml_dtypes.bfloat16
    npair = heads_per_core // 2
    CH = C // 128
    in_maps = []
    for core in range(ncores):
        b, g = core // 2, core % 2
        xT = np.ascontiguousarray(x[b].T).reshape(CH, 128, T)
        wq2 = np.empty((npair, 128, CH * 2 * D), np.float32)
        wk2 = np.empty_like(wq2)
        wv2 = np.empty_like(wq2)
        bq2 = np.empty((npair, 2 * D), np.float32)
        bk2 = np.empty_like(bq2)
        bv2 = np.zeros((npair, 2 * D), np.float32)
        for p in range(npair):
            ha = g * heads_per_core + 2 * p
            r0 = ha * D
            for dst, off in ((wq2, 0), (wk2, C), (wv2, 2 * C)):
                wpair = w_attn[off + r0: off + r0 + 2 * D, :]       # [128, C]
                dst[p] = (wpair.T.reshape(CH, 128, 2 * D)
                          .transpose(1, 0, 2).reshape(128, CH * 2 * D))
            bq2[p] = b_attn[r0: r0 + 2 * D]
            bk2[p] = b_attn[C + r0: C + r0 + 2 * D]
            bv2[p] = b_attn[2 * C + r0: 2 * C + r0 + 2 * D]
        cols = slice(g * heads_per_core * D, (g + 1) * heads_per_core * D)
        wpT = (np.ascontiguousarray(w_proj[:, cols].T)
               .reshape(npair, 128, w_proj.shape[0]))
        b2 = np.empty((2 * D, 2 * npair), np.float32)
        for p in range(npair):
            b2[:, 2 * p] = bq2[p]
            b2[:, 2 * p + 1] = bk2[p]
        m = {
            "xT": xT.astype(bf16), "wq2": wq2.astype(bf16),
            "wk2": wk2.astype(bf16), "wv2": wv2.astype(bf16),
            "wpT": np.ascontiguousarray(wpT).astype(bf16),
            "b2": b2, "bvr": bv2.reshape(1, npair * 2 * D).copy(),
        }
        in_maps.append(m)
    return in_maps


_NC_CACHE = {}


def _get_nc(mmdt_name="bf16"):
    if mmdt_name not in _NC_CACHE:
        _NC_CACHE[mmdt_name] = build_attn_nc(
            T=T, C=C, NPAIR=NPAIR, COUT=C, D=D, TQ=512, mmdt_name=mmdt_name)
    return _NC_CACHE[mmdt_name]


MMDT = "bf16"


def kernel(x, w_attn, b_attn, w_proj, b_proj):
    _, _, _, _, bass_utils = _import_concourse()
    x = np.asarray(x, np.float32)
    w_attn = np.asarray(w_attn, np.float32)
    b_attn = np.asarray(b_attn, np.float32)
    w_proj = np.asarray(w_proj, np.float32)
    b_proj = np.asarray(b_proj, np.float32)

    nc = _get_nc(MMDT)
    in_maps = shard_inputs(x, w_attn, b_attn, w_proj, mmdt_name=MMDT)
    res = bass_utils.run_bass_kernel_spmd(nc, in_maps, core_ids=list(range(NCORES)))
    out = np.empty((B, T, C), np.float32)
    for b in range(B):
        out[b] = (res.results[2 * b]["out"].astype(np.float32)
                  + res.results[2 * b + 1]["out"].astype(np.float32) + b_proj)
    return out
